# revision 59
# baseline (speedup 1.0000x reference)
"""GCN layer (gather + scale + segment-sum + linear + relu) on 8 TRN2 cores.

Sharding: each core owns a contiguous range of 6250 dst nodes and processes
every edge pointing into that range (edge lists are grouped by dst on the
host — pure format work, like building a CSR). Cores are fully independent:
no collectives.

Device pipeline per core (single phase, gather-bound):
  1. The gather table is the raw input: 256 B row per PAIR of nodes
     [featE(48 f16) | featO(48) | degE | degO | pad], so gather indices
     (src>>1) fit in signed int16 and no scaled table is ever built in
     DRAM. Per 128-dst-node block, dma_gather (SWDGE -> 16 SDMA engines)
     pulls the per-edge rows into SBUF.
  2. Per-slot X' = feat * rsqrt(max(out_deg,1)) on DVE, using the degrees
     that ride in each gathered row; only the parity half each tile
     actually reads is scaled.
  3. Transposed one-hot segment-sum on TensorE: psT[feat, node] +=
     msg^T @ one-hot, so no PE transpose is needed before the linear.
     Main slots (rank<16 per (node, src-parity)) share one constant
     one-hot rhs; overflow edges use per-lane target ids expanded on DVE
     to one-hot masks via is_equal against an iota constant (2 B/lane
     instead of full masks in the block stream). A full-width level-2
     matmul opens/closes the PSUM accumulation (start/stop act on whole
     partition rows).
  4. h^T = psT * rsqrt(max(in_deg,1)) straight into SBUF (fused
     PSUM-drain + scale); zero-in-degree fallback only compiled when such
     nodes exist. Linear + biased relu run interleaved every 4 blocks and
     the transposed output is written out per 512-column chunk.
Host concatenates + transposes the 8 output slices.
"""

import numpy as np

N = 50000
E = 1600000
D = 48
NCORES = 8
NPC = 6250             # nodes per core
BLOCKS = 49            # node range padded to 49*128 = 6272
NPAD = BLOCKS * 128
PAIRS = 25088          # rows in the pair table (incl. zero rows)
ZPAIR = 25000          # an all-zero pair row used for padding slots
WMAIN = 16             # main slots per (node, parity)
GROUPS = 4             # 32-node groups per block
OVG_T = 1              # level-1 overflow tiles per group

_CACHE = {}


# ---------------------------------------------------------------------------
# Host-side preprocessing: dtype narrowing, edge grouping by dst, slot
# assignment, one-hot mask construction, layout reshapes. All value math
# (rsqrt, scaling, sums, linear) runs on device.
# ---------------------------------------------------------------------------

def _host_prep(features, src, dst):
    src = np.asarray(src).astype(np.int64)
    dst = np.asarray(dst).astype(np.int64)
    feats = np.asarray(features, dtype=np.float32)

    par = (src & 1).astype(np.int64)
    pair = (src >> 1).astype(np.int64)
    out_deg = np.bincount(src, minlength=N).astype(np.int32)
    in_deg = np.bincount(dst, minlength=N).astype(np.int32)

    # fp16 gather table: 256 B row per pair of nodes, feature halves plus the
    # raw out-degrees (small ints are exact in fp16) packed into the pad
    # bytes. The device gathers rows per edge and applies rsqrt(max(deg,1))
    # scaling per slot, so no scaled table is ever materialized in DRAM.
    xrow = np.zeros((PAIRS, 128), dtype=np.float16)
    xrow[: N // 2, 0:48] = feats[0::2]
    xrow[: N // 2, 48:96] = feats[1::2]
    xrow[: N // 2, 96] = out_deg[0::2]
    xrow[: N // 2, 97] = out_deg[1::2]

    # rank of each edge within its (dst, parity) bucket
    key = dst * 2 + par
    sort2 = np.argsort(key, kind="stable")
    ks = key[sort2]
    runstart = np.r_[0, np.flatnonzero(np.diff(ks)) + 1]
    runid = np.zeros(E, np.int64)
    runid[runstart] = 1
    runid = np.cumsum(runid) - 1
    rank = np.empty(E, np.int64)
    rank[sort2] = np.arange(E) - runstart[runid]

    core = dst // NPC
    nl = dst - core * NPC
    block = nl // 128
    v = nl % 128
    g = v // 32

    # ---- main slots (rank < WMAIN) --------------------------------------
    # slot: tile T = par*16 + g*4 + rank//4, lane = (v%32)*4 + rank%4
    selm = rank < WMAIN
    Tm = par[selm] * WMAIN + g[selm] * 4 + rank[selm] // 4
    lanem = (v[selm] % 32) * 4 + rank[selm] % 4

    # ---- overflow (rank >= WMAIN): level-1 per (core,block,group) -------
    selo = ~selm
    okey = (core[selo] * BLOCKS + block[selo]) * GROUPS + g[selo]
    osort = np.argsort(okey, kind="stable")
    oks = okey[osort]
    orunstart = np.r_[0, np.flatnonzero(np.diff(oks)) + 1]
    orunid = np.zeros(len(oks), np.int64)
    orunid[orunstart] = 1
    orunid = np.cumsum(orunid) - 1
    q = np.empty(len(oks), np.int64)
    q[osort] = np.arange(len(oks)) - orunstart[orunid]

    lvl1 = q < OVG_T * 128
    # ---- level-2: leftovers per (core, block) ---------------------------
    sel2 = ~lvl1
    oidx = np.flatnonzero(selo)
    e2 = oidx[sel2]
    k2 = core[e2] * BLOCKS + block[e2]
    s2 = np.argsort(k2, kind="stable")
    k2s = k2[s2]
    if len(k2s):
        rs2 = np.r_[0, np.flatnonzero(np.diff(k2s)) + 1]
        rid2 = np.zeros(len(k2s), np.int64)
        rid2[rs2] = 1
        rid2 = np.cumsum(rid2) - 1
        q2 = np.empty(len(k2s), np.int64)
        q2[s2] = np.arange(len(k2s)) - rs2[rid2]
        OV2_T = max(1, int(np.ceil((q2.max() + 1) / 128)))
    else:
        q2 = np.zeros(0, np.int64)
        OV2_T = 1
    assert OV2_T <= 4, f"unexpectedly deep level-2 overflow: {OV2_T}"

    TILES = 2 * WMAIN + GROUPS * OVG_T + OV2_T
    NIDX = TILES * 128

    gidx = np.full((NCORES, BLOCKS, TILES, 128), ZPAIR, np.int32)
    # per-lane scatter targets for the overflow tiles (-1 = inactive); the
    # device expands them to one-hot matmul masks with an is_equal against
    # an iota constant, so only 2 B/lane/tile ride in the block stream
    tg1 = np.full((NCORES, BLOCKS, GROUPS * OVG_T, 2, 128), -1, np.float16)
    tg2 = np.full((NCORES, BLOCKS, OV2_T, 2, 128), -1, np.float16)

    gidx[core[selm], block[selm], Tm, lanem] = pair[selm]

    e1 = oidx[lvl1]
    t1 = q[lvl1] // 128
    lane1 = q[lvl1] % 128
    T1 = 2 * WMAIN + g[e1] * OVG_T + t1
    gidx[core[e1], block[e1], T1, lane1] = pair[e1]
    tg1[core[e1], block[e1], g[e1] * OVG_T + t1, par[e1], lane1] = v[e1] % 32

    t2 = q2 // 128
    lane2 = q2 % 128
    T2 = 2 * WMAIN + GROUPS * OVG_T + t2
    gidx[core[e2], block[e2], T2, lane2] = pair[e2]
    tg2[core[e2], block[e2], t2, par[e2], lane2] = v[e2]

    # Pad slots (unused main/overflow lanes) all point at ZPAIR, the all-zero
    # row, so every block gathers exactly NIDX valid rows and num_idxs_reg is
    # the compile-time constant NIDX — no per-block count registers.
    iso = bool((in_deg == 0).any())  # any zero-in-degree node anywhere
    per_core = []
    for c in range(NCORES):
        flat = gidx[c].reshape(BLOCKS, NIDX).astype(np.int16)
        wrapped = flat.reshape(BLOCKS, NIDX // 16, 16).transpose(0, 2, 1)
        gidx_w = np.broadcast_to(
            wrapped[:, None, :, :], (BLOCKS, 8, 16, NIDX // 16)
        ).reshape(BLOCKS, 128, NIDX // 16).copy()

        tg1_dev = np.ascontiguousarray(
            tg1[c].transpose(0, 3, 1, 2).reshape(
                BLOCKS, 128, GROUPS * OVG_T * 2))
        tg2_dev = np.ascontiguousarray(
            tg2[c].transpose(0, 3, 1, 2).reshape(BLOCKS, 128, OV2_T * 2))

        nlo = c * NPC
        ind = np.zeros(NPAD, np.uint8)
        assert in_deg.max() <= 255
        ind[:NPC] = in_deg[nlo:nlo + NPC]
        # transposed + replicated across the 48 feature partitions so the
        # device can compute cj in the [feat, node] layout the blend uses
        indegT_dev = np.ascontiguousarray(
            np.broadcast_to(ind[None, :], (D, NPAD)))

        blk = np.concatenate([
            gidx_w.view(np.uint8).reshape(BLOCKS, 128, -1),
            tg1_dev.view(np.uint8).reshape(BLOCKS, 128, -1),
            tg2_dev.view(np.uint8).reshape(BLOCKS, 128, -1),
        ], axis=2)
        entry = dict(blk=np.ascontiguousarray(blk), indegT=indegT_dev)
        if iso:
            fc = np.zeros((NPAD, D), np.float32)
            fc[:NPC] = feats[nlo:nlo + NPC]
            entry["featT"] = np.ascontiguousarray(fc.T)
        per_core.append(entry)

    meta = dict(OV2_T=OV2_T, TILES=TILES, NIDX=NIDX, iso=iso)
    return xrow, per_core, meta


# ---------------------------------------------------------------------------
# Device program
# ---------------------------------------------------------------------------

def _build_program(meta):
    import concourse.tile as tile
    from concourse import bacc, mybir

    OV2_T, TILES, NIDX = meta["OV2_T"], meta["TILES"], meta["NIDX"]
    iso = meta["iso"]
    f16 = mybir.dt.float16
    f32 = mybir.dt.float32
    i16 = mybir.dt.int16
    AF = mybir.ActivationFunctionType
    OP = mybir.AluOpType

    nc = bacc.Bacc("TRN2", target_bir_lowering=False, debug=False,
                   num_devices=NCORES, num_swdge_queues=2)

    xrow = nc.dram_tensor("xrow", [PAIRS, 128], f16, kind="ExternalInput").ap()
    GIB = (NIDX // 16) * 2
    NT1 = GROUPS * OVG_T * 2
    NT2 = OV2_T * 2
    MGB = NT1 * 2
    M2B = NT2 * 2
    BLKB = GIB + MGB + M2B
    u8 = mybir.dt.uint8
    blkD = nc.dram_tensor("blk", [BLOCKS, 128, BLKB], u8, kind="ExternalInput").ap()
    indegTD = nc.dram_tensor("indegT", [D, NPAD], u8, kind="ExternalInput").ap()
    wbD = nc.dram_tensor("wb", [D, D], f32, kind="ExternalInput").ap()
    biasD = nc.dram_tensor("bias", [D, 1], f32, kind="ExternalInput").ap()
    id32D = nc.dram_tensor("id32", [128, GROUPS * 32], f16, kind="ExternalInput").ap()
    iotaD = nc.dram_tensor("iota", [128, NT1 * 32 + NT2 * 128], f16,
                           kind="ExternalInput").ap()
    if iso:
        featTD = nc.dram_tensor("featT", [D, NPAD], f32,
                                kind="ExternalInput").ap()
    outD = nc.dram_tensor("out", [D, NPAD], f16, kind="ExternalOutput").ap()

    with tile.TileContext(nc) as tc:
        with tc.tile_pool(name="const", bufs=1) as cpool, \
             tc.tile_pool(name="big", bufs=1) as bigpool:

            wb_s = cpool.tile([D, D], f32, tag="wb")
            nc.scalar.dma_start(out=wb_s[:], in_=wbD)
            bias_s = cpool.tile([D, 1], f32, tag="bias")
            nc.scalar.dma_start(out=bias_s[:], in_=biasD)
            id32 = cpool.tile([128, GROUPS * 32], f16, tag="id32")
            nc.scalar.dma_start(out=id32[:], in_=id32D)
            iota = cpool.tile([128, NT1 * 32 + NT2 * 128], f16, tag="iota")
            nc.scalar.dma_start(out=iota[:], in_=iotaD)
            iot1 = iota[:, 0:NT1 * 32].rearrange("p (t j) -> p t j", t=NT1)
            iot2 = iota[:, NT1 * 32:].rearrange("p (t j) -> p t j", t=NT2)

            # cj in the transposed [feat, node] layout the blend uses;
            # in-degree arrives pre-replicated across the 48 feature rows
            cjT = bigpool.tile([D, NPAD], f32, tag="cjT")
            ind_s = bigpool.tile([D, NPAD], u8, tag="indT")
            nc.scalar.dma_start(out=ind_s[:], in_=indegTD)
            nc.vector.tensor_scalar_max(cjT[:], ind_s[:], 1.0)
            nc.scalar.activation(cjT[:], cjT[:], AF.Sqrt)
            nc.vector.reciprocal(cjT[:], cjT[:])
            if iso:
                mask = bigpool.tile([D, NPAD], f32, tag="mask")
                nc.vector.tensor_scalar(mask[:], ind_s[:], 0.0, None,
                                        OP.is_gt)
                nc.vector.tensor_mul(cjT[:], cjT[:], mask[:])
                featT_s = bigpool.tile([D, NPAD], f32, tag="featT")
                nc.sync.dma_start(out=featT_s[:], in_=featTD)
                fbT = bigpool.tile([D, NPAD], f32, tag="fbT")
                nc.vector.tensor_scalar(mask[:], mask[:], -1.0, 1.0,
                                        OP.mult, OP.add)  # 1 - mask
                nc.vector.tensor_mul(fbT[:], featT_s[:], mask[:])

            hT1 = bigpool.tile([D, BLOCKS * 128], f32, tag="hT1")

            # ---- gather + transposed segment-sum over 49 blocks ----------
            with tc.tile_pool(name="blk", bufs=8) as blkpool, \
                 tc.tile_pool(name="msg", bufs=5) as msgpool, \
                 tc.tile_pool(name="sm", bufs=4) as smpool, \
                 tc.tile_pool(name="ps", bufs=4, space="PSUM") as pspool, \
                 tc.tile_pool(name="aux", bufs=3, space="PSUM") as auxpool:

                CH = 512
                outT = bigpool.tile([D, BLOCKS * 128], f16, tag="outT")

                for b in range(BLOCKS):
                    blkt = blkpool.tile([128, BLKB], u8, tag="blkt")
                    nc.sync.dma_start(out=blkt[:], in_=blkD[b])
                    gi = blkt[:, 0:GIB].bitcast(i16)
                    tg1 = blkt[:, GIB:GIB + MGB].bitcast(f16)
                    tg2 = blkt[:, GIB + MGB:BLKB].bitcast(f16)

                    msg = msgpool.tile([128, TILES, 128], f16, tag="msg")
                    nc.gpsimd.dma_gather(
                        out_ap=msg[:],
                        in_ap=xrow,
                        idxs_ap=gi,
                        num_idxs=NIDX,
                        num_idxs_reg=NIDX,
                        elem_size=128,
                        queue_num=b % 2,
                        single_packet=False,
                    )

                    # per-slot X' = feat * rsqrt(max(out_deg, 1)); degrees
                    # for both parities ride in cols 96:98 of each row.
                    # Main tiles are parity-pure, so only the half the
                    # matmuls actually read gets scaled; overflow tiles
                    # (both halves read) get both.
                    sc32 = smpool.tile([128, TILES, 2], f32, tag="sc32")
                    nc.vector.tensor_scalar_max(
                        sc32[:], msg[:, :, 96:98], 1.0)
                    nc.scalar.activation(sc32[:], sc32[:], AF.Sqrt)
                    nc.vector.reciprocal(sc32[:], sc32[:])
                    sc = smpool.tile([128, TILES, 2], f16, tag="sc")
                    nc.vector.tensor_copy(sc[:], sc32[:])

                    # expand per-lane targets into one-hot matmul masks
                    oh1 = smpool.tile([128, NT1, 32], f16, tag="oh1")
                    nc.vector.tensor_tensor(
                        oh1[:], iot1,
                        tg1.unsqueeze(2).to_broadcast([128, NT1, 32]),
                        OP.is_equal)
                    oh2 = smpool.tile([128, NT2, 128], f16, tag="oh2")
                    nc.vector.tensor_tensor(
                        oh2[:], iot2,
                        tg2.unsqueeze(2).to_broadcast([128, NT2, 128]),
                        OP.is_equal)
                    W2 = 2 * WMAIN
                    # overflow tiles first: the level-2 opener matmul only
                    # needs those, so the PE stream starts while the main
                    # halves are still being scaled
                    for t0, t1, p, c0 in ((W2, TILES, 0, 0),
                                          (W2, TILES, 1, 48),
                                          (0, WMAIN, 0, 0),
                                          (WMAIN, W2, 1, 48)):
                        nt = t1 - t0
                        nc.vector.tensor_tensor(
                            msg[:, t0:t1, c0:c0 + 48],
                            msg[:, t0:t1, c0:c0 + 48],
                            sc[:, t0:t1, p:p + 1].to_broadcast([128, nt, 48]),
                            OP.mult)

                    # transposed segment-sum: psT[feat, node] += msg^T @ 1hot
                    # PSUM start/stop act on whole partition rows, so the
                    # full-width level-2 matmuls open (start) and close
                    # (stop) the accumulation; everything else accumulates
                    # in between on 32-col slices.
                    psT = pspool.tile([D, 128], f32, tag="psT")
                    T2a = 2 * WMAIN + GROUPS * OVG_T
                    nc.tensor.matmul(
                        psT[:, :], lhsT=msg[:, T2a, 0:D],
                        rhs=oh2[:, 0, :],
                        start=True, stop=False, skip_group_check=True)
                    for p, c0 in ((0, 0), (1, 48)):
                        for gg in range(GROUPS):
                            for j in range(4):
                                T = p * WMAIN + gg * 4 + j
                                nc.tensor.matmul(
                                    psT[:, 32 * gg:32 * (gg + 1)],
                                    lhsT=msg[:, T, c0:c0 + D],
                                    rhs=id32[:, 32 * gg:32 * (gg + 1)],
                                    start=False, stop=False,
                                    skip_group_check=True)
                    for gg in range(GROUPS):
                        for t in range(OVG_T):
                            T = 2 * WMAIN + gg * OVG_T + t
                            for p, c0 in ((0, 0), (1, 48)):
                                nc.tensor.matmul(
                                    psT[:, 32 * gg:32 * (gg + 1)],
                                    lhsT=msg[:, T, c0:c0 + D],
                                    rhs=oh1[:, (gg * OVG_T + t) * 2 + p, :],
                                    start=False, stop=False,
                                    skip_group_check=True)
                    for t in range(OV2_T):
                        T = T2a + t
                        for pi, (p, c0) in enumerate(((0, 0), (1, 48))):
                            if t == 0 and pi == 0:
                                continue  # issued above as the opener
                            last = (t == OV2_T - 1) and (pi == 1)
                            nc.tensor.matmul(
                                psT[:, :],
                                lhsT=msg[:, T, c0:c0 + D],
                                rhs=oh2[:, t * 2 + p, :],
                                start=False, stop=last, skip_group_check=True)

                    # h^T = agg^T * cj (+ feat^T on zero-in-degree nodes)
                    bsl = slice(b * 128, (b + 1) * 128)
                    if iso:
                        th = smpool.tile([D, 128], f32, tag="th")
                        nc.vector.tensor_tensor(th[:], psT[:], cjT[:, bsl],
                                                OP.mult)
                        nc.vector.tensor_add(hT1[:, bsl], th[:],
                                             fbT[:, bsl])
                    else:
                        nc.vector.tensor_tensor(hT1[:, bsl], psT[:],
                                                cjT[:, bsl], OP.mult)

                    # ---- linear + relu, interleaved per 4 blocks ---------
                    if b % 4 == 3 or b == BLOCKS - 1:
                        lo = (b // 4) * CH
                        hi = (b + 1) * 128
                        po = auxpool.tile([D, CH], f32, tag="aux")
                        nc.tensor.matmul(po[:, 0:hi - lo], lhsT=wb_s[:],
                                         rhs=hT1[:, lo:hi], start=True,
                                         stop=True)
                        nc.scalar.activation(outT[:, lo:hi], po[:, 0:hi - lo],
                                             AF.Relu, bias=bias_s[:, 0:1])
                        nc.sync.dma_start(out=outD[:, lo:hi],
                                          in_=outT[:, lo:hi])

                # output stays transposed [48, NPAD], written per chunk
                # inside the loop; host transposes back

    nc.compile()
    return nc


# ---------------------------------------------------------------------------
# Entry point
# ---------------------------------------------------------------------------

def kernel(features, src, dst, W, b):
    from concourse.bass_utils import run_bass_kernel_spmd

    xrow, per_core, meta = _host_prep(features, src, dst)

    key = (meta["OV2_T"], meta["iso"])
    if key not in _CACHE:
        _CACHE[key] = _build_program(meta)
    nc = _CACHE[key]

    Wb = np.ascontiguousarray(np.asarray(W, np.float32).T)
    bias = np.ascontiguousarray(np.asarray(b, np.float32)[:, None])
    id32 = np.zeros((128, GROUPS * 32), np.float16)
    lanes = np.arange(128)
    for gg in range(GROUPS):
        id32[lanes, gg * 32 + lanes // 4] = 1.0
    nt1 = GROUPS * OVG_T * 2
    nt2 = meta["OV2_T"] * 2
    iorow = np.concatenate([
        np.tile(np.arange(32, dtype=np.float16), nt1),
        np.tile(np.arange(128, dtype=np.float16), nt2)])
    iota = np.ascontiguousarray(
        np.broadcast_to(iorow[None, :], (128, iorow.size)))

    in_maps = []
    for c in range(NCORES):
        pc = per_core[c]
        m = {
            "xrow": xrow, "blk": pc["blk"], "indegT": pc["indegT"],
            "wb": Wb, "bias": bias, "id32": id32, "iota": iota,
        }
        if meta["iso"]:
            m["featT"] = pc["featT"]
        in_maps.append(m)

    res = run_bass_kernel_spmd(nc, in_maps, core_ids=list(range(NCORES)))
    globals()["LAST_RESULTS"] = res
    out = np.concatenate(
        [res.results[c]["out"][:, :NPC].T for c in range(NCORES)], axis=0)
    return np.ascontiguousarray(out, dtype=np.float32)



# revision 60
# speedup vs baseline: 1.0037x; 1.0037x over previous
"""GCN layer (gather + scale + segment-sum + linear + relu) on 8 TRN2 cores.

Sharding: each core owns a contiguous range of 6250 dst nodes and processes
every edge pointing into that range (edge lists are grouped by dst on the
host — pure format work, like building a CSR). Cores are fully independent:
no collectives.

Device pipeline per core (single phase, gather-bound):
  1. The gather table is the raw input: 256 B row per PAIR of nodes
     [featE(48 f16) | featO(48) | degE | degO | pad], so gather indices
     (src>>1) fit in signed int16 and no scaled table is ever built in
     DRAM. Per 128-dst-node block, dma_gather (SWDGE -> 16 SDMA engines)
     pulls the per-edge rows into SBUF.
  2. Per-slot X' = feat * rsqrt(max(out_deg,1)) on DVE, using the degrees
     that ride in each gathered row; only the parity half each tile
     actually reads is scaled.
  3. Transposed one-hot segment-sum on TensorE: psT[feat, node] +=
     msg^T @ one-hot, so no PE transpose is needed before the linear.
     Main slots (rank<16 per (node, src-parity)) share one constant
     one-hot rhs; overflow edges use per-lane target ids expanded on DVE
     to one-hot masks via is_equal against an iota constant (2 B/lane
     instead of full masks in the block stream). A full-width level-2
     matmul opens/closes the PSUM accumulation (start/stop act on whole
     partition rows).
  4. h^T = psT * rsqrt(max(in_deg,1)) straight into SBUF (fused
     PSUM-drain + scale); zero-in-degree fallback only compiled when such
     nodes exist. Linear + biased relu run interleaved every 4 blocks and
     the transposed output is written out per 512-column chunk.
Host concatenates + transposes the 8 output slices.
"""

import numpy as np

N = 50000
E = 1600000
D = 48
NCORES = 8
NPC = 6250             # nodes per core
BLOCKS = 49            # node range padded to 49*128 = 6272
NPAD = BLOCKS * 128
PAIRS = 25088          # rows in the pair table (incl. zero rows)
ZPAIR = 25000          # an all-zero pair row used for padding slots
WMAIN = 16             # main slots per (node, parity)
GROUPS = 4             # 32-node groups per block
OVG_T = 1              # level-1 overflow tiles per group

_CACHE = {}


# ---------------------------------------------------------------------------
# Host-side preprocessing: dtype narrowing, edge grouping by dst, slot
# assignment, one-hot mask construction, layout reshapes. All value math
# (rsqrt, scaling, sums, linear) runs on device.
# ---------------------------------------------------------------------------

def _host_prep(features, src, dst):
    src = np.asarray(src).astype(np.int64)
    dst = np.asarray(dst).astype(np.int64)
    feats = np.asarray(features, dtype=np.float32)

    par = (src & 1).astype(np.int64)
    pair = (src >> 1).astype(np.int64)
    out_deg = np.bincount(src, minlength=N).astype(np.int32)
    in_deg = np.bincount(dst, minlength=N).astype(np.int32)

    # fp16 gather table: 256 B row per pair of nodes, feature halves plus the
    # raw out-degrees (small ints are exact in fp16) packed into the pad
    # bytes. The device gathers rows per edge and applies rsqrt(max(deg,1))
    # scaling per slot, so no scaled table is ever materialized in DRAM.
    xrow = np.zeros((PAIRS, 128), dtype=np.float16)
    xrow[: N // 2, 0:48] = feats[0::2]
    xrow[: N // 2, 48:96] = feats[1::2]
    xrow[: N // 2, 96] = out_deg[0::2]
    xrow[: N // 2, 97] = out_deg[1::2]
    xrow[N // 2:, 96:98] = 1.0  # synthetic pad rows: rsqrt(1) = 1
    # every gathered slot's used half has deg >= 1 (an edge implies its
    # source has out-degree >= 1), so the clamp is only needed if some
    # node has out-degree 0 yet is a neighbor via the other parity
    nzdeg = not bool((out_deg == 0).any())

    # rank of each edge within its (dst, parity) bucket
    key = dst * 2 + par
    sort2 = np.argsort(key, kind="stable")
    ks = key[sort2]
    runstart = np.r_[0, np.flatnonzero(np.diff(ks)) + 1]
    runid = np.zeros(E, np.int64)
    runid[runstart] = 1
    runid = np.cumsum(runid) - 1
    rank = np.empty(E, np.int64)
    rank[sort2] = np.arange(E) - runstart[runid]

    core = dst // NPC
    nl = dst - core * NPC
    block = nl // 128
    v = nl % 128
    g = v // 32

    # ---- main slots (rank < WMAIN) --------------------------------------
    # slot: tile T = par*16 + g*4 + rank//4, lane = (v%32)*4 + rank%4
    selm = rank < WMAIN
    Tm = par[selm] * WMAIN + g[selm] * 4 + rank[selm] // 4
    lanem = (v[selm] % 32) * 4 + rank[selm] % 4

    # ---- overflow (rank >= WMAIN): level-1 per (core,block,group) -------
    selo = ~selm
    okey = (core[selo] * BLOCKS + block[selo]) * GROUPS + g[selo]
    osort = np.argsort(okey, kind="stable")
    oks = okey[osort]
    orunstart = np.r_[0, np.flatnonzero(np.diff(oks)) + 1]
    orunid = np.zeros(len(oks), np.int64)
    orunid[orunstart] = 1
    orunid = np.cumsum(orunid) - 1
    q = np.empty(len(oks), np.int64)
    q[osort] = np.arange(len(oks)) - orunstart[orunid]

    lvl1 = q < OVG_T * 128
    # ---- level-2: leftovers per (core, block) ---------------------------
    sel2 = ~lvl1
    oidx = np.flatnonzero(selo)
    e2 = oidx[sel2]
    k2 = core[e2] * BLOCKS + block[e2]
    s2 = np.argsort(k2, kind="stable")
    k2s = k2[s2]
    if len(k2s):
        rs2 = np.r_[0, np.flatnonzero(np.diff(k2s)) + 1]
        rid2 = np.zeros(len(k2s), np.int64)
        rid2[rs2] = 1
        rid2 = np.cumsum(rid2) - 1
        q2 = np.empty(len(k2s), np.int64)
        q2[s2] = np.arange(len(k2s)) - rs2[rid2]
        OV2_T = max(1, int(np.ceil((q2.max() + 1) / 128)))
    else:
        q2 = np.zeros(0, np.int64)
        OV2_T = 1
    assert OV2_T <= 4, f"unexpectedly deep level-2 overflow: {OV2_T}"

    TILES = 2 * WMAIN + GROUPS * OVG_T + OV2_T
    NIDX = TILES * 128

    gidx = np.full((NCORES, BLOCKS, TILES, 128), ZPAIR, np.int32)
    # per-lane scatter targets for the overflow tiles (-1 = inactive); the
    # device expands them to one-hot matmul masks with an is_equal against
    # an iota constant, so only 2 B/lane/tile ride in the block stream
    tg1 = np.full((NCORES, BLOCKS, GROUPS * OVG_T, 2, 128), -1, np.float16)
    tg2 = np.full((NCORES, BLOCKS, OV2_T, 2, 128), -1, np.float16)

    gidx[core[selm], block[selm], Tm, lanem] = pair[selm]

    e1 = oidx[lvl1]
    t1 = q[lvl1] // 128
    lane1 = q[lvl1] % 128
    T1 = 2 * WMAIN + g[e1] * OVG_T + t1
    gidx[core[e1], block[e1], T1, lane1] = pair[e1]
    tg1[core[e1], block[e1], g[e1] * OVG_T + t1, par[e1], lane1] = v[e1] % 32

    t2 = q2 // 128
    lane2 = q2 % 128
    T2 = 2 * WMAIN + GROUPS * OVG_T + t2
    gidx[core[e2], block[e2], T2, lane2] = pair[e2]
    tg2[core[e2], block[e2], t2, par[e2], lane2] = v[e2]

    # Pad slots (unused main/overflow lanes) all point at ZPAIR, the all-zero
    # row, so every block gathers exactly NIDX valid rows and num_idxs_reg is
    # the compile-time constant NIDX — no per-block count registers.
    iso = bool((in_deg == 0).any())  # any zero-in-degree node anywhere
    per_core = []
    for c in range(NCORES):
        flat = gidx[c].reshape(BLOCKS, NIDX).astype(np.int16)
        wrapped = flat.reshape(BLOCKS, NIDX // 16, 16).transpose(0, 2, 1)
        gidx_w = np.broadcast_to(
            wrapped[:, None, :, :], (BLOCKS, 8, 16, NIDX // 16)
        ).reshape(BLOCKS, 128, NIDX // 16).copy()

        tg1_dev = np.ascontiguousarray(
            tg1[c].transpose(0, 3, 1, 2).reshape(
                BLOCKS, 128, GROUPS * OVG_T * 2))
        tg2_dev = np.ascontiguousarray(
            tg2[c].transpose(0, 3, 1, 2).reshape(BLOCKS, 128, OV2_T * 2))

        nlo = c * NPC
        ind = np.zeros(NPAD, np.uint8)
        assert in_deg.max() <= 255
        ind[:NPC] = in_deg[nlo:nlo + NPC]
        # transposed + replicated across the 48 feature partitions so the
        # device can compute cj in the [feat, node] layout the blend uses
        indegT_dev = np.ascontiguousarray(
            np.broadcast_to(ind[None, :], (D, NPAD)))

        blk = np.concatenate([
            gidx_w.view(np.uint8).reshape(BLOCKS, 128, -1),
            tg1_dev.view(np.uint8).reshape(BLOCKS, 128, -1),
            tg2_dev.view(np.uint8).reshape(BLOCKS, 128, -1),
        ], axis=2)
        entry = dict(blk=np.ascontiguousarray(blk), indegT=indegT_dev)
        if iso:
            fc = np.zeros((NPAD, D), np.float32)
            fc[:NPC] = feats[nlo:nlo + NPC]
            entry["featT"] = np.ascontiguousarray(fc.T)
        per_core.append(entry)

    meta = dict(OV2_T=OV2_T, TILES=TILES, NIDX=NIDX, iso=iso, nzdeg=nzdeg)
    return xrow, per_core, meta


# ---------------------------------------------------------------------------
# Device program
# ---------------------------------------------------------------------------

def _build_program(meta):
    import concourse.tile as tile
    from concourse import bacc, mybir

    OV2_T, TILES, NIDX = meta["OV2_T"], meta["TILES"], meta["NIDX"]
    iso = meta["iso"]
    f16 = mybir.dt.float16
    f32 = mybir.dt.float32
    i16 = mybir.dt.int16
    AF = mybir.ActivationFunctionType
    OP = mybir.AluOpType

    nc = bacc.Bacc("TRN2", target_bir_lowering=False, debug=False,
                   num_devices=NCORES, num_swdge_queues=2)

    xrow = nc.dram_tensor("xrow", [PAIRS, 128], f16, kind="ExternalInput").ap()
    GIB = (NIDX // 16) * 2
    NT1 = GROUPS * OVG_T * 2
    NT2 = OV2_T * 2
    MGB = NT1 * 2
    M2B = NT2 * 2
    BLKB = GIB + MGB + M2B
    u8 = mybir.dt.uint8
    blkD = nc.dram_tensor("blk", [BLOCKS, 128, BLKB], u8, kind="ExternalInput").ap()
    indegTD = nc.dram_tensor("indegT", [D, NPAD], u8, kind="ExternalInput").ap()
    wbD = nc.dram_tensor("wb", [D, D], f32, kind="ExternalInput").ap()
    biasD = nc.dram_tensor("bias", [D, 1], f32, kind="ExternalInput").ap()
    id32D = nc.dram_tensor("id32", [128, GROUPS * 32], f16, kind="ExternalInput").ap()
    iotaD = nc.dram_tensor("iota", [128, NT1 * 32 + NT2 * 128], f16,
                           kind="ExternalInput").ap()
    if iso:
        featTD = nc.dram_tensor("featT", [D, NPAD], f32,
                                kind="ExternalInput").ap()
    outD = nc.dram_tensor("out", [D, NPAD], f16, kind="ExternalOutput").ap()

    with tile.TileContext(nc) as tc:
        with tc.tile_pool(name="const", bufs=1) as cpool, \
             tc.tile_pool(name="big", bufs=1) as bigpool:

            wb_s = cpool.tile([D, D], f32, tag="wb")
            nc.scalar.dma_start(out=wb_s[:], in_=wbD)
            bias_s = cpool.tile([D, 1], f32, tag="bias")
            nc.scalar.dma_start(out=bias_s[:], in_=biasD)
            id32 = cpool.tile([128, GROUPS * 32], f16, tag="id32")
            nc.scalar.dma_start(out=id32[:], in_=id32D)
            iota = cpool.tile([128, NT1 * 32 + NT2 * 128], f16, tag="iota")
            nc.scalar.dma_start(out=iota[:], in_=iotaD)
            iot1 = iota[:, 0:NT1 * 32].rearrange("p (t j) -> p t j", t=NT1)
            iot2 = iota[:, NT1 * 32:].rearrange("p (t j) -> p t j", t=NT2)

            # cj in the transposed [feat, node] layout the blend uses;
            # in-degree arrives pre-replicated across the 48 feature rows
            cjT = bigpool.tile([D, NPAD], f32, tag="cjT")
            ind_s = bigpool.tile([D, NPAD], u8, tag="indT")
            nc.scalar.dma_start(out=ind_s[:], in_=indegTD)
            nc.vector.tensor_scalar_max(cjT[:], ind_s[:], 1.0)
            nc.scalar.activation(cjT[:], cjT[:], AF.Sqrt)
            nc.vector.reciprocal(cjT[:], cjT[:])
            if iso:
                mask = bigpool.tile([D, NPAD], f32, tag="mask")
                nc.vector.tensor_scalar(mask[:], ind_s[:], 0.0, None,
                                        OP.is_gt)
                nc.vector.tensor_mul(cjT[:], cjT[:], mask[:])
                featT_s = bigpool.tile([D, NPAD], f32, tag="featT")
                nc.sync.dma_start(out=featT_s[:], in_=featTD)
                fbT = bigpool.tile([D, NPAD], f32, tag="fbT")
                nc.vector.tensor_scalar(mask[:], mask[:], -1.0, 1.0,
                                        OP.mult, OP.add)  # 1 - mask
                nc.vector.tensor_mul(fbT[:], featT_s[:], mask[:])

            hT1 = bigpool.tile([D, BLOCKS * 128], f32, tag="hT1")

            # ---- gather + transposed segment-sum over 49 blocks ----------
            with tc.tile_pool(name="blk", bufs=8) as blkpool, \
                 tc.tile_pool(name="msg", bufs=5) as msgpool, \
                 tc.tile_pool(name="sm", bufs=4) as smpool, \
                 tc.tile_pool(name="ps", bufs=4, space="PSUM") as pspool, \
                 tc.tile_pool(name="aux", bufs=3, space="PSUM") as auxpool:

                CH = 512
                outT = bigpool.tile([D, BLOCKS * 128], f16, tag="outT")

                for b in range(BLOCKS):
                    blkt = blkpool.tile([128, BLKB], u8, tag="blkt")
                    nc.sync.dma_start(out=blkt[:], in_=blkD[b])
                    gi = blkt[:, 0:GIB].bitcast(i16)
                    tg1 = blkt[:, GIB:GIB + MGB].bitcast(f16)
                    tg2 = blkt[:, GIB + MGB:BLKB].bitcast(f16)

                    msg = msgpool.tile([128, TILES, 128], f16, tag="msg")
                    nc.gpsimd.dma_gather(
                        out_ap=msg[:],
                        in_ap=xrow,
                        idxs_ap=gi,
                        num_idxs=NIDX,
                        num_idxs_reg=NIDX,
                        elem_size=128,
                        queue_num=b % 2,
                        single_packet=False,
                    )

                    # per-slot X' = feat * rsqrt(max(out_deg, 1)); degrees
                    # for both parities ride in cols 96:98 of each row.
                    # Main tiles are parity-pure, so only the half the
                    # matmuls actually read gets scaled; overflow tiles
                    # (both halves read) get both.
                    sc32 = smpool.tile([128, TILES, 2], f32, tag="sc32")
                    if meta["nzdeg"]:
                        nc.scalar.activation(sc32[:], msg[:, :, 96:98],
                                             AF.Sqrt)
                    else:
                        nc.vector.tensor_scalar_max(
                            sc32[:], msg[:, :, 96:98], 1.0)
                        nc.scalar.activation(sc32[:], sc32[:], AF.Sqrt)
                    nc.vector.reciprocal(sc32[:], sc32[:])
                    sc = smpool.tile([128, TILES, 2], f16, tag="sc")
                    nc.vector.tensor_copy(sc[:], sc32[:])

                    # expand per-lane targets into one-hot matmul masks
                    oh1 = smpool.tile([128, NT1, 32], f16, tag="oh1")
                    nc.vector.tensor_tensor(
                        oh1[:], iot1,
                        tg1.unsqueeze(2).to_broadcast([128, NT1, 32]),
                        OP.is_equal)
                    oh2 = smpool.tile([128, NT2, 128], f16, tag="oh2")
                    nc.vector.tensor_tensor(
                        oh2[:], iot2,
                        tg2.unsqueeze(2).to_broadcast([128, NT2, 128]),
                        OP.is_equal)
                    W2 = 2 * WMAIN
                    # overflow tiles first: the level-2 opener matmul only
                    # needs those, so the PE stream starts while the main
                    # halves are still being scaled
                    for t0, t1, p, c0 in ((W2, TILES, 0, 0),
                                          (W2, TILES, 1, 48),
                                          (0, WMAIN, 0, 0),
                                          (WMAIN, W2, 1, 48)):
                        nt = t1 - t0
                        nc.vector.tensor_tensor(
                            msg[:, t0:t1, c0:c0 + 48],
                            msg[:, t0:t1, c0:c0 + 48],
                            sc[:, t0:t1, p:p + 1].to_broadcast([128, nt, 48]),
                            OP.mult)

                    # transposed segment-sum: psT[feat, node] += msg^T @ 1hot
                    # PSUM start/stop act on whole partition rows, so the
                    # full-width level-2 matmuls open (start) and close
                    # (stop) the accumulation; everything else accumulates
                    # in between on 32-col slices.
                    psT = pspool.tile([D, 128], f32, tag="psT")
                    T2a = 2 * WMAIN + GROUPS * OVG_T
                    nc.tensor.matmul(
                        psT[:, :], lhsT=msg[:, T2a, 0:D],
                        rhs=oh2[:, 0, :],
                        start=True, stop=False, skip_group_check=True)
                    for p, c0 in ((0, 0), (1, 48)):
                        for gg in range(GROUPS):
                            for j in range(4):
                                T = p * WMAIN + gg * 4 + j
                                nc.tensor.matmul(
                                    psT[:, 32 * gg:32 * (gg + 1)],
                                    lhsT=msg[:, T, c0:c0 + D],
                                    rhs=id32[:, 32 * gg:32 * (gg + 1)],
                                    start=False, stop=False,
                                    skip_group_check=True)
                    for gg in range(GROUPS):
                        for t in range(OVG_T):
                            T = 2 * WMAIN + gg * OVG_T + t
                            for p, c0 in ((0, 0), (1, 48)):
                                nc.tensor.matmul(
                                    psT[:, 32 * gg:32 * (gg + 1)],
                                    lhsT=msg[:, T, c0:c0 + D],
                                    rhs=oh1[:, (gg * OVG_T + t) * 2 + p, :],
                                    start=False, stop=False,
                                    skip_group_check=True)
                    for t in range(OV2_T):
                        T = T2a + t
                        for pi, (p, c0) in enumerate(((0, 0), (1, 48))):
                            if t == 0 and pi == 0:
                                continue  # issued above as the opener
                            last = (t == OV2_T - 1) and (pi == 1)
                            nc.tensor.matmul(
                                psT[:, :],
                                lhsT=msg[:, T, c0:c0 + D],
                                rhs=oh2[:, t * 2 + p, :],
                                start=False, stop=last, skip_group_check=True)

                    # h^T = agg^T * cj (+ feat^T on zero-in-degree nodes)
                    bsl = slice(b * 128, (b + 1) * 128)
                    if iso:
                        th = smpool.tile([D, 128], f32, tag="th")
                        nc.vector.tensor_tensor(th[:], psT[:], cjT[:, bsl],
                                                OP.mult)
                        nc.vector.tensor_add(hT1[:, bsl], th[:],
                                             fbT[:, bsl])
                    else:
                        nc.vector.tensor_tensor(hT1[:, bsl], psT[:],
                                                cjT[:, bsl], OP.mult)

                    # ---- linear + relu, interleaved per 4 blocks ---------
                    if b % 4 == 3 or b == BLOCKS - 1:
                        lo = (b // 4) * CH
                        hi = (b + 1) * 128
                        po = auxpool.tile([D, CH], f32, tag="aux")
                        nc.tensor.matmul(po[:, 0:hi - lo], lhsT=wb_s[:],
                                         rhs=hT1[:, lo:hi], start=True,
                                         stop=True)
                        nc.scalar.activation(outT[:, lo:hi], po[:, 0:hi - lo],
                                             AF.Relu, bias=bias_s[:, 0:1])
                        nc.sync.dma_start(out=outD[:, lo:hi],
                                          in_=outT[:, lo:hi])

                # output stays transposed [48, NPAD], written per chunk
                # inside the loop; host transposes back

    nc.compile()
    return nc


# ---------------------------------------------------------------------------
# Entry point
# ---------------------------------------------------------------------------

def kernel(features, src, dst, W, b):
    from concourse.bass_utils import run_bass_kernel_spmd

    xrow, per_core, meta = _host_prep(features, src, dst)

    key = (meta["OV2_T"], meta["iso"], meta["nzdeg"])
    if key not in _CACHE:
        _CACHE[key] = _build_program(meta)
    nc = _CACHE[key]

    Wb = np.ascontiguousarray(np.asarray(W, np.float32).T)
    bias = np.ascontiguousarray(np.asarray(b, np.float32)[:, None])
    id32 = np.zeros((128, GROUPS * 32), np.float16)
    lanes = np.arange(128)
    for gg in range(GROUPS):
        id32[lanes, gg * 32 + lanes // 4] = 1.0
    nt1 = GROUPS * OVG_T * 2
    nt2 = meta["OV2_T"] * 2
    iorow = np.concatenate([
        np.tile(np.arange(32, dtype=np.float16), nt1),
        np.tile(np.arange(128, dtype=np.float16), nt2)])
    iota = np.ascontiguousarray(
        np.broadcast_to(iorow[None, :], (128, iorow.size)))

    in_maps = []
    for c in range(NCORES):
        pc = per_core[c]
        m = {
            "xrow": xrow, "blk": pc["blk"], "indegT": pc["indegT"],
            "wb": Wb, "bias": bias, "id32": id32, "iota": iota,
        }
        if meta["iso"]:
            m["featT"] = pc["featT"]
        in_maps.append(m)

    res = run_bass_kernel_spmd(nc, in_maps, core_ids=list(range(NCORES)))
    globals()["LAST_RESULTS"] = res
    out = np.concatenate(
        [res.results[c]["out"][:, :NPC].T for c in range(NCORES)], axis=0)
    return np.ascontiguousarray(out, dtype=np.float32)



# revision 61
# speedup vs baseline: 1.0043x; 1.0005x over previous
"""GCN layer (gather + scale + segment-sum + linear + relu) on 8 TRN2 cores.

Sharding: each core owns a contiguous range of 6250 dst nodes and processes
every edge pointing into that range (edge lists are grouped by dst on the
host — pure format work, like building a CSR). Cores are fully independent:
no collectives.

Device pipeline per core (single phase, gather-bound):
  1. The gather table is the raw input: 256 B row per PAIR of nodes
     [featE(48 f16) | featO(48) | degE | degO | pad], so gather indices
     (src>>1) fit in signed int16 and no scaled table is ever built in
     DRAM. Per 128-dst-node block, dma_gather (SWDGE -> 16 SDMA engines)
     pulls the per-edge rows into SBUF.
  2. Per-slot X' = feat * rsqrt(max(out_deg,1)) on DVE, using the degrees
     that ride in each gathered row; only the parity half each tile
     actually reads is scaled.
  3. Transposed one-hot segment-sum on TensorE: psT[feat, node] +=
     msg^T @ one-hot, so no PE transpose is needed before the linear.
     Main slots (rank<16 per (node, src-parity)) share one constant
     one-hot rhs; overflow edges use per-lane target ids expanded on DVE
     to one-hot masks via is_equal against an iota constant (2 B/lane
     instead of full masks in the block stream). A full-width level-2
     matmul opens/closes the PSUM accumulation (start/stop act on whole
     partition rows).
  4. h^T = psT * rsqrt(max(in_deg,1)) straight into SBUF (fused
     PSUM-drain + scale); zero-in-degree fallback only compiled when such
     nodes exist. Linear + biased relu run interleaved every 4 blocks and
     the transposed output is written out per 512-column chunk.
Host concatenates + transposes the 8 output slices.
"""

import numpy as np

N = 50000
E = 1600000
D = 48
NCORES = 8
NPC = 6250             # nodes per core
BLOCKS = 49            # node range padded to 49*128 = 6272
NPAD = BLOCKS * 128
PAIRS = 25088          # rows in the pair table (incl. zero rows)
ZPAIR = 25000          # an all-zero pair row used for padding slots
WMAIN = 16             # main slots per (node, parity)
GROUPS = 4             # 32-node groups per block
OVG_T = 1              # level-1 overflow tiles per group

_CACHE = {}


# ---------------------------------------------------------------------------
# Host-side preprocessing: dtype narrowing, edge grouping by dst, slot
# assignment, one-hot mask construction, layout reshapes. All value math
# (rsqrt, scaling, sums, linear) runs on device.
# ---------------------------------------------------------------------------

def _host_prep(features, src, dst):
    src = np.asarray(src).astype(np.int64)
    dst = np.asarray(dst).astype(np.int64)
    feats = np.asarray(features, dtype=np.float32)

    par = (src & 1).astype(np.int64)
    pair = (src >> 1).astype(np.int64)
    out_deg = np.bincount(src, minlength=N).astype(np.int32)
    in_deg = np.bincount(dst, minlength=N).astype(np.int32)

    # fp16 gather table: 256 B row per pair of nodes, feature halves plus the
    # raw out-degrees (small ints are exact in fp16) packed into the pad
    # bytes. The device gathers rows per edge and applies rsqrt(max(deg,1))
    # scaling per slot, so no scaled table is ever materialized in DRAM.
    xrow = np.zeros((PAIRS, 128), dtype=np.float16)
    xrow[: N // 2, 0:48] = feats[0::2]
    xrow[: N // 2, 48:96] = feats[1::2]
    xrow[: N // 2, 96] = out_deg[0::2]
    xrow[: N // 2, 97] = out_deg[1::2]
    xrow[N // 2:, 96:98] = 1.0  # synthetic pad rows: rsqrt(1) = 1
    # every gathered slot's used half has deg >= 1 (an edge implies its
    # source has out-degree >= 1), so the clamp is only needed if some
    # node has out-degree 0 yet is a neighbor via the other parity
    nzdeg = not bool((out_deg == 0).any())

    # rank of each edge within its (dst, parity) bucket
    key = dst * 2 + par
    sort2 = np.argsort(key, kind="stable")
    ks = key[sort2]
    runstart = np.r_[0, np.flatnonzero(np.diff(ks)) + 1]
    runid = np.zeros(E, np.int64)
    runid[runstart] = 1
    runid = np.cumsum(runid) - 1
    rank = np.empty(E, np.int64)
    rank[sort2] = np.arange(E) - runstart[runid]

    core = dst // NPC
    nl = dst - core * NPC
    block = nl // 128
    v = nl % 128
    g = v // 32

    # ---- main slots (rank < WMAIN) --------------------------------------
    # slot: tile T = par*16 + g*4 + rank//4, lane = (v%32)*4 + rank%4
    selm = rank < WMAIN
    Tm = par[selm] * WMAIN + g[selm] * 4 + rank[selm] // 4
    lanem = (v[selm] % 32) * 4 + rank[selm] % 4

    # ---- overflow (rank >= WMAIN): level-1 per (core,block,group) -------
    selo = ~selm
    okey = (core[selo] * BLOCKS + block[selo]) * GROUPS + g[selo]
    osort = np.argsort(okey, kind="stable")
    oks = okey[osort]
    orunstart = np.r_[0, np.flatnonzero(np.diff(oks)) + 1]
    orunid = np.zeros(len(oks), np.int64)
    orunid[orunstart] = 1
    orunid = np.cumsum(orunid) - 1
    q = np.empty(len(oks), np.int64)
    q[osort] = np.arange(len(oks)) - orunstart[orunid]

    lvl1 = q < OVG_T * 128
    # ---- level-2: leftovers per (core, block) ---------------------------
    sel2 = ~lvl1
    oidx = np.flatnonzero(selo)
    e2 = oidx[sel2]
    k2 = core[e2] * BLOCKS + block[e2]
    s2 = np.argsort(k2, kind="stable")
    k2s = k2[s2]
    if len(k2s):
        rs2 = np.r_[0, np.flatnonzero(np.diff(k2s)) + 1]
        rid2 = np.zeros(len(k2s), np.int64)
        rid2[rs2] = 1
        rid2 = np.cumsum(rid2) - 1
        q2 = np.empty(len(k2s), np.int64)
        q2[s2] = np.arange(len(k2s)) - rs2[rid2]
        OV2_T = max(1, int(np.ceil((q2.max() + 1) / 128)))
    else:
        q2 = np.zeros(0, np.int64)
        OV2_T = 1
    assert OV2_T <= 4, f"unexpectedly deep level-2 overflow: {OV2_T}"

    TILES = 2 * WMAIN + GROUPS * OVG_T + OV2_T
    NIDX = TILES * 128

    gidx = np.full((NCORES, BLOCKS, TILES, 128), ZPAIR, np.int32)
    # per-lane scatter targets for the overflow tiles (-1 = inactive); the
    # device expands them to one-hot matmul masks with an is_equal against
    # an iota constant, so only 2 B/lane/tile ride in the block stream
    tg1 = np.full((NCORES, BLOCKS, GROUPS * OVG_T, 2, 128), -1, np.float16)
    tg2 = np.full((NCORES, BLOCKS, OV2_T, 2, 128), -1, np.float16)

    gidx[core[selm], block[selm], Tm, lanem] = pair[selm]

    e1 = oidx[lvl1]
    t1 = q[lvl1] // 128
    lane1 = q[lvl1] % 128
    T1 = 2 * WMAIN + g[e1] * OVG_T + t1
    gidx[core[e1], block[e1], T1, lane1] = pair[e1]
    tg1[core[e1], block[e1], g[e1] * OVG_T + t1, par[e1], lane1] = v[e1] % 32

    t2 = q2 // 128
    lane2 = q2 % 128
    T2 = 2 * WMAIN + GROUPS * OVG_T + t2
    gidx[core[e2], block[e2], T2, lane2] = pair[e2]
    tg2[core[e2], block[e2], t2, par[e2], lane2] = v[e2]

    # Pad slots (unused main/overflow lanes) all point at ZPAIR, the all-zero
    # row, so every block gathers exactly NIDX valid rows and num_idxs_reg is
    # the compile-time constant NIDX — no per-block count registers.
    iso = bool((in_deg == 0).any())  # any zero-in-degree node anywhere
    per_core = []
    for c in range(NCORES):
        flat = gidx[c].reshape(BLOCKS, NIDX).astype(np.int16)
        wrapped = flat.reshape(BLOCKS, NIDX // 16, 16).transpose(0, 2, 1)
        gidx_w = np.broadcast_to(
            wrapped[:, None, :, :], (BLOCKS, 8, 16, NIDX // 16)
        ).reshape(BLOCKS, 128, NIDX // 16).copy()

        tg1_dev = np.ascontiguousarray(
            tg1[c].transpose(0, 3, 1, 2).reshape(
                BLOCKS, 128, GROUPS * OVG_T * 2))
        tg2_dev = np.ascontiguousarray(
            tg2[c].transpose(0, 3, 1, 2).reshape(BLOCKS, 128, OV2_T * 2))

        nlo = c * NPC
        ind = np.zeros(NPAD, np.uint8)
        assert in_deg.max() <= 255
        ind[:NPC] = in_deg[nlo:nlo + NPC]
        # transposed + replicated across the 48 feature partitions so the
        # device can compute cj in the [feat, node] layout the blend uses
        indegT_dev = np.ascontiguousarray(
            np.broadcast_to(ind[None, :], (D, NPAD)))

        blk = np.concatenate([
            gidx_w.view(np.uint8).reshape(BLOCKS, 128, -1),
            tg1_dev.view(np.uint8).reshape(BLOCKS, 128, -1),
            tg2_dev.view(np.uint8).reshape(BLOCKS, 128, -1),
        ], axis=2)
        entry = dict(blk=np.ascontiguousarray(blk), indegT=indegT_dev)
        if iso:
            fc = np.zeros((NPAD, D), np.float32)
            fc[:NPC] = feats[nlo:nlo + NPC]
            entry["featT"] = np.ascontiguousarray(fc.T)
        per_core.append(entry)

    meta = dict(OV2_T=OV2_T, TILES=TILES, NIDX=NIDX, iso=iso, nzdeg=nzdeg)
    return xrow, per_core, meta


# ---------------------------------------------------------------------------
# Device program
# ---------------------------------------------------------------------------

def _build_program(meta):
    import concourse.tile as tile
    from concourse import bacc, mybir

    OV2_T, TILES, NIDX = meta["OV2_T"], meta["TILES"], meta["NIDX"]
    iso = meta["iso"]
    f16 = mybir.dt.float16
    f32 = mybir.dt.float32
    i16 = mybir.dt.int16
    AF = mybir.ActivationFunctionType
    OP = mybir.AluOpType

    nc = bacc.Bacc("TRN2", target_bir_lowering=False, debug=False,
                   num_devices=NCORES, num_swdge_queues=2)

    xrow = nc.dram_tensor("xrow", [PAIRS, 128], f16, kind="ExternalInput").ap()
    GIB = (NIDX // 16) * 2
    NT1 = GROUPS * OVG_T * 2
    NT2 = OV2_T * 2
    MGB = NT1 * 2
    M2B = NT2 * 2
    BLKB = GIB + MGB + M2B
    u8 = mybir.dt.uint8
    blkD = nc.dram_tensor("blk", [BLOCKS, 128, BLKB], u8, kind="ExternalInput").ap()
    indegTD = nc.dram_tensor("indegT", [D, NPAD], u8, kind="ExternalInput").ap()
    wbD = nc.dram_tensor("wb", [D, D], f32, kind="ExternalInput").ap()
    biasD = nc.dram_tensor("bias", [D, 1], f32, kind="ExternalInput").ap()
    id32D = nc.dram_tensor("id32", [128, GROUPS * 32], f16, kind="ExternalInput").ap()
    iotaD = nc.dram_tensor("iota", [128, NT1 * 32 + NT2 * 128], f16,
                           kind="ExternalInput").ap()
    if iso:
        featTD = nc.dram_tensor("featT", [D, NPAD], f32,
                                kind="ExternalInput").ap()
    outD = nc.dram_tensor("out", [D, NPAD], f16, kind="ExternalOutput").ap()

    with tile.TileContext(nc) as tc:
        with tc.tile_pool(name="const", bufs=1) as cpool, \
             tc.tile_pool(name="big", bufs=1) as bigpool:

            wb_s = cpool.tile([D, D], f32, tag="wb")
            nc.scalar.dma_start(out=wb_s[:], in_=wbD)
            bias_s = cpool.tile([D, 1], f32, tag="bias")
            nc.scalar.dma_start(out=bias_s[:], in_=biasD)
            id32 = cpool.tile([128, GROUPS * 32], f16, tag="id32")
            nc.scalar.dma_start(out=id32[:], in_=id32D)
            iota = cpool.tile([128, NT1 * 32 + NT2 * 128], f16, tag="iota")
            nc.scalar.dma_start(out=iota[:], in_=iotaD)
            iot1 = iota[:, 0:NT1 * 32].rearrange("p (t j) -> p t j", t=NT1)
            iot2 = iota[:, NT1 * 32:].rearrange("p (t j) -> p t j", t=NT2)

            # cj in the transposed [feat, node] layout the blend uses;
            # in-degree arrives pre-replicated across the 48 feature rows
            cjT = bigpool.tile([D, NPAD], f32, tag="cjT")
            ind_s = bigpool.tile([D, NPAD], u8, tag="indT")
            nc.scalar.dma_start(out=ind_s[:], in_=indegTD)
            nc.vector.tensor_scalar_max(cjT[:], ind_s[:], 1.0)
            nc.scalar.activation(cjT[:], cjT[:], AF.Sqrt)
            nc.vector.reciprocal(cjT[:], cjT[:])
            if iso:
                mask = bigpool.tile([D, NPAD], f32, tag="mask")
                nc.vector.tensor_scalar(mask[:], ind_s[:], 0.0, None,
                                        OP.is_gt)
                nc.vector.tensor_mul(cjT[:], cjT[:], mask[:])
                featT_s = bigpool.tile([D, NPAD], f32, tag="featT")
                nc.sync.dma_start(out=featT_s[:], in_=featTD)
                fbT = bigpool.tile([D, NPAD], f32, tag="fbT")
                nc.vector.tensor_scalar(mask[:], mask[:], -1.0, 1.0,
                                        OP.mult, OP.add)  # 1 - mask
                nc.vector.tensor_mul(fbT[:], featT_s[:], mask[:])

            hT1 = bigpool.tile([D, BLOCKS * 128], f32, tag="hT1")

            # ---- gather + transposed segment-sum over 49 blocks ----------
            with tc.tile_pool(name="blk", bufs=8) as blkpool, \
                 tc.tile_pool(name="msg", bufs=5) as msgpool, \
                 tc.tile_pool(name="sm", bufs=4) as smpool, \
                 tc.tile_pool(name="ps", bufs=4, space="PSUM") as pspool, \
                 tc.tile_pool(name="aux", bufs=3, space="PSUM") as auxpool:

                CH = 512
                outT = bigpool.tile([D, BLOCKS * 128], f16, tag="outT")

                for b in range(BLOCKS):
                    blkt = blkpool.tile([128, BLKB], u8, tag="blkt")
                    nc.sync.dma_start(out=blkt[:], in_=blkD[b])
                    gi = blkt[:, 0:GIB].bitcast(i16)
                    tg1 = blkt[:, GIB:GIB + MGB].bitcast(f16)
                    tg2 = blkt[:, GIB + MGB:BLKB].bitcast(f16)

                    msg = msgpool.tile([128, TILES, 128], f16, tag="msg")
                    nc.gpsimd.dma_gather(
                        out_ap=msg[:],
                        in_ap=xrow,
                        idxs_ap=gi,
                        num_idxs=NIDX,
                        num_idxs_reg=NIDX,
                        elem_size=128,
                        queue_num=b % 2,
                        single_packet=False,
                    )

                    # per-slot X' = feat * rsqrt(max(out_deg, 1)); degrees
                    # for both parities ride in cols 96:98 of each row.
                    # Main tiles are parity-pure, so only the half the
                    # matmuls actually read gets scaled; overflow tiles
                    # (both halves read) get both.
                    sc32 = smpool.tile([128, TILES, 2], f32, tag="sc32")
                    if meta["nzdeg"]:
                        nc.scalar.activation(sc32[:], msg[:, :, 96:98],
                                             AF.Sqrt)
                    else:
                        nc.vector.tensor_scalar_max(
                            sc32[:], msg[:, :, 96:98], 1.0)
                        nc.scalar.activation(sc32[:], sc32[:], AF.Sqrt)
                    sc = smpool.tile([128, TILES, 2], f16, tag="sc")
                    with nc.allow_low_precision("rsqrt scale, values <= 1"):
                        nc.vector.reciprocal(sc[:], sc32[:])

                    # expand per-lane targets into one-hot matmul masks
                    oh1 = smpool.tile([128, NT1, 32], f16, tag="oh1")
                    nc.vector.tensor_tensor(
                        oh1[:], iot1,
                        tg1.unsqueeze(2).to_broadcast([128, NT1, 32]),
                        OP.is_equal)
                    oh2 = smpool.tile([128, NT2, 128], f16, tag="oh2")
                    nc.vector.tensor_tensor(
                        oh2[:], iot2,
                        tg2.unsqueeze(2).to_broadcast([128, NT2, 128]),
                        OP.is_equal)
                    W2 = 2 * WMAIN
                    # overflow tiles first: the level-2 opener matmul only
                    # needs those, so the PE stream starts while the main
                    # halves are still being scaled
                    for t0, t1, p, c0 in ((W2, TILES, 0, 0),
                                          (W2, TILES, 1, 48),
                                          (0, WMAIN, 0, 0),
                                          (WMAIN, W2, 1, 48)):
                        nt = t1 - t0
                        nc.vector.tensor_tensor(
                            msg[:, t0:t1, c0:c0 + 48],
                            msg[:, t0:t1, c0:c0 + 48],
                            sc[:, t0:t1, p:p + 1].to_broadcast([128, nt, 48]),
                            OP.mult)

                    # transposed segment-sum: psT[feat, node] += msg^T @ 1hot
                    # PSUM start/stop act on whole partition rows, so the
                    # full-width level-2 matmuls open (start) and close
                    # (stop) the accumulation; everything else accumulates
                    # in between on 32-col slices.
                    psT = pspool.tile([D, 128], f32, tag="psT")
                    T2a = 2 * WMAIN + GROUPS * OVG_T
                    nc.tensor.matmul(
                        psT[:, :], lhsT=msg[:, T2a, 0:D],
                        rhs=oh2[:, 0, :],
                        start=True, stop=False, skip_group_check=True)
                    for p, c0 in ((0, 0), (1, 48)):
                        for gg in range(GROUPS):
                            for j in range(4):
                                T = p * WMAIN + gg * 4 + j
                                nc.tensor.matmul(
                                    psT[:, 32 * gg:32 * (gg + 1)],
                                    lhsT=msg[:, T, c0:c0 + D],
                                    rhs=id32[:, 32 * gg:32 * (gg + 1)],
                                    start=False, stop=False,
                                    skip_group_check=True)
                    for gg in range(GROUPS):
                        for t in range(OVG_T):
                            T = 2 * WMAIN + gg * OVG_T + t
                            for p, c0 in ((0, 0), (1, 48)):
                                nc.tensor.matmul(
                                    psT[:, 32 * gg:32 * (gg + 1)],
                                    lhsT=msg[:, T, c0:c0 + D],
                                    rhs=oh1[:, (gg * OVG_T + t) * 2 + p, :],
                                    start=False, stop=False,
                                    skip_group_check=True)
                    for t in range(OV2_T):
                        T = T2a + t
                        for pi, (p, c0) in enumerate(((0, 0), (1, 48))):
                            if t == 0 and pi == 0:
                                continue  # issued above as the opener
                            last = (t == OV2_T - 1) and (pi == 1)
                            nc.tensor.matmul(
                                psT[:, :],
                                lhsT=msg[:, T, c0:c0 + D],
                                rhs=oh2[:, t * 2 + p, :],
                                start=False, stop=last, skip_group_check=True)

                    # h^T = agg^T * cj (+ feat^T on zero-in-degree nodes)
                    bsl = slice(b * 128, (b + 1) * 128)
                    if iso:
                        th = smpool.tile([D, 128], f32, tag="th")
                        nc.vector.tensor_tensor(th[:], psT[:], cjT[:, bsl],
                                                OP.mult)
                        nc.vector.tensor_add(hT1[:, bsl], th[:],
                                             fbT[:, bsl])
                    else:
                        nc.vector.tensor_tensor(hT1[:, bsl], psT[:],
                                                cjT[:, bsl], OP.mult)

                    # ---- linear + relu, interleaved per 4 blocks ---------
                    if b % 4 == 3 or b == BLOCKS - 1:
                        lo = (b // 4) * CH
                        hi = (b + 1) * 128
                        po = auxpool.tile([D, CH], f32, tag="aux")
                        nc.tensor.matmul(po[:, 0:hi - lo], lhsT=wb_s[:],
                                         rhs=hT1[:, lo:hi], start=True,
                                         stop=True)
                        nc.scalar.activation(outT[:, lo:hi], po[:, 0:hi - lo],
                                             AF.Relu, bias=bias_s[:, 0:1])
                        nc.sync.dma_start(out=outD[:, lo:hi],
                                          in_=outT[:, lo:hi])

                # output stays transposed [48, NPAD], written per chunk
                # inside the loop; host transposes back

    nc.compile()
    return nc


# ---------------------------------------------------------------------------
# Entry point
# ---------------------------------------------------------------------------

def kernel(features, src, dst, W, b):
    from concourse.bass_utils import run_bass_kernel_spmd

    xrow, per_core, meta = _host_prep(features, src, dst)

    key = (meta["OV2_T"], meta["iso"], meta["nzdeg"])
    if key not in _CACHE:
        _CACHE[key] = _build_program(meta)
    nc = _CACHE[key]

    Wb = np.ascontiguousarray(np.asarray(W, np.float32).T)
    bias = np.ascontiguousarray(np.asarray(b, np.float32)[:, None])
    id32 = np.zeros((128, GROUPS * 32), np.float16)
    lanes = np.arange(128)
    for gg in range(GROUPS):
        id32[lanes, gg * 32 + lanes // 4] = 1.0
    nt1 = GROUPS * OVG_T * 2
    nt2 = meta["OV2_T"] * 2
    iorow = np.concatenate([
        np.tile(np.arange(32, dtype=np.float16), nt1),
        np.tile(np.arange(128, dtype=np.float16), nt2)])
    iota = np.ascontiguousarray(
        np.broadcast_to(iorow[None, :], (128, iorow.size)))

    in_maps = []
    for c in range(NCORES):
        pc = per_core[c]
        m = {
            "xrow": xrow, "blk": pc["blk"], "indegT": pc["indegT"],
            "wb": Wb, "bias": bias, "id32": id32, "iota": iota,
        }
        if meta["iso"]:
            m["featT"] = pc["featT"]
        in_maps.append(m)

    res = run_bass_kernel_spmd(nc, in_maps, core_ids=list(range(NCORES)))
    globals()["LAST_RESULTS"] = res
    out = np.concatenate(
        [res.results[c]["out"][:, :NPC].T for c in range(NCORES)], axis=0)
    return np.ascontiguousarray(out, dtype=np.float32)



# revision 62
# speedup vs baseline: 1.0044x; 1.0001x over previous
"""GCN layer (gather + scale + segment-sum + linear + relu) on 8 TRN2 cores.

Sharding: each core owns a contiguous range of 6250 dst nodes and processes
every edge pointing into that range (edge lists are grouped by dst on the
host — pure format work, like building a CSR). Cores are fully independent:
no collectives.

Device pipeline per core (single phase, gather-bound):
  1. The gather table is the raw input: 256 B row per PAIR of nodes
     [featE(48 f16) | featO(48) | degE | degO | pad], so gather indices
     (src>>1) fit in signed int16 and no scaled table is ever built in
     DRAM. Per 128-dst-node block, dma_gather (SWDGE -> 16 SDMA engines)
     pulls the per-edge rows into SBUF.
  2. Per-slot X' = feat * rsqrt(max(out_deg,1)) on DVE, using the degrees
     that ride in each gathered row; only the parity half each tile
     actually reads is scaled.
  3. Transposed one-hot segment-sum on TensorE: psT[feat, node] +=
     msg^T @ one-hot, so no PE transpose is needed before the linear.
     Main slots (rank<16 per (node, src-parity)) share one constant
     one-hot rhs; overflow edges use per-lane target ids expanded on DVE
     to one-hot masks via is_equal against an iota constant (2 B/lane
     instead of full masks in the block stream). A full-width level-2
     matmul opens/closes the PSUM accumulation (start/stop act on whole
     partition rows).
  4. h^T = psT * rsqrt(max(in_deg,1)) straight into SBUF (fused
     PSUM-drain + scale); zero-in-degree fallback only compiled when such
     nodes exist. Linear + biased relu run interleaved every 4 blocks and
     the transposed output is written out per 512-column chunk.
Host concatenates + transposes the 8 output slices.
"""

import numpy as np

N = 50000
E = 1600000
D = 48
NCORES = 8
NPC = 6250             # nodes per core
BLOCKS = 49            # node range padded to 49*128 = 6272
NPAD = BLOCKS * 128
PAIRS = 25088          # rows in the pair table (incl. zero rows)
ZPAIR = 25000          # an all-zero pair row used for padding slots
WMAIN = 16             # main slots per (node, parity)
GROUPS = 4             # 32-node groups per block
OVG_T = 1              # level-1 overflow tiles per group

_CACHE = {}


# ---------------------------------------------------------------------------
# Host-side preprocessing: dtype narrowing, edge grouping by dst, slot
# assignment, one-hot mask construction, layout reshapes. All value math
# (rsqrt, scaling, sums, linear) runs on device.
# ---------------------------------------------------------------------------

def _host_prep(features, src, dst):
    src = np.asarray(src).astype(np.int64)
    dst = np.asarray(dst).astype(np.int64)
    feats = np.asarray(features, dtype=np.float32)

    par = (src & 1).astype(np.int64)
    pair = (src >> 1).astype(np.int64)
    out_deg = np.bincount(src, minlength=N).astype(np.int32)
    in_deg = np.bincount(dst, minlength=N).astype(np.int32)

    # fp16 gather table: 256 B row per pair of nodes, feature halves plus the
    # raw out-degrees (small ints are exact in fp16) packed into the pad
    # bytes. The device gathers rows per edge and applies rsqrt(max(deg,1))
    # scaling per slot, so no scaled table is ever materialized in DRAM.
    xrow = np.zeros((PAIRS, 128), dtype=np.float16)
    xrow[: N // 2, 0:48] = feats[0::2]
    xrow[: N // 2, 48:96] = feats[1::2]
    xrow[: N // 2, 96] = out_deg[0::2]
    xrow[: N // 2, 97] = out_deg[1::2]
    xrow[N // 2:, 96:98] = 1.0  # synthetic pad rows: rsqrt(1) = 1
    # every gathered slot's used half has deg >= 1 (an edge implies its
    # source has out-degree >= 1), so the clamp is only needed if some
    # node has out-degree 0 yet is a neighbor via the other parity
    nzdeg = not bool((out_deg == 0).any())

    # rank of each edge within its (dst, parity) bucket
    key = dst * 2 + par
    sort2 = np.argsort(key, kind="stable")
    ks = key[sort2]
    runstart = np.r_[0, np.flatnonzero(np.diff(ks)) + 1]
    runid = np.zeros(E, np.int64)
    runid[runstart] = 1
    runid = np.cumsum(runid) - 1
    rank = np.empty(E, np.int64)
    rank[sort2] = np.arange(E) - runstart[runid]

    core = dst // NPC
    nl = dst - core * NPC
    block = nl // 128
    v = nl % 128
    g = v // 32

    # ---- main slots (rank < WMAIN) --------------------------------------
    # slot: tile T = par*16 + g*4 + rank//4, lane = (v%32)*4 + rank%4
    selm = rank < WMAIN
    Tm = par[selm] * WMAIN + g[selm] * 4 + rank[selm] // 4
    lanem = (v[selm] % 32) * 4 + rank[selm] % 4

    # ---- overflow (rank >= WMAIN): level-1 per (core,block,group) -------
    selo = ~selm
    okey = (core[selo] * BLOCKS + block[selo]) * GROUPS + g[selo]
    osort = np.argsort(okey, kind="stable")
    oks = okey[osort]
    orunstart = np.r_[0, np.flatnonzero(np.diff(oks)) + 1]
    orunid = np.zeros(len(oks), np.int64)
    orunid[orunstart] = 1
    orunid = np.cumsum(orunid) - 1
    q = np.empty(len(oks), np.int64)
    q[osort] = np.arange(len(oks)) - orunstart[orunid]

    lvl1 = q < OVG_T * 128
    # ---- level-2: leftovers per (core, block) ---------------------------
    sel2 = ~lvl1
    oidx = np.flatnonzero(selo)
    e2 = oidx[sel2]
    k2 = core[e2] * BLOCKS + block[e2]
    s2 = np.argsort(k2, kind="stable")
    k2s = k2[s2]
    if len(k2s):
        rs2 = np.r_[0, np.flatnonzero(np.diff(k2s)) + 1]
        rid2 = np.zeros(len(k2s), np.int64)
        rid2[rs2] = 1
        rid2 = np.cumsum(rid2) - 1
        q2 = np.empty(len(k2s), np.int64)
        q2[s2] = np.arange(len(k2s)) - rs2[rid2]
        OV2_T = max(1, int(np.ceil((q2.max() + 1) / 128)))
    else:
        q2 = np.zeros(0, np.int64)
        OV2_T = 1
    assert OV2_T <= 4, f"unexpectedly deep level-2 overflow: {OV2_T}"

    TILES = 2 * WMAIN + GROUPS * OVG_T + OV2_T
    NIDX = TILES * 128

    gidx = np.full((NCORES, BLOCKS, TILES, 128), ZPAIR, np.int32)
    # per-lane scatter targets for the overflow tiles (-1 = inactive); the
    # device expands them to one-hot matmul masks with an is_equal against
    # an iota constant, so only 2 B/lane/tile ride in the block stream
    tg1 = np.full((NCORES, BLOCKS, GROUPS * OVG_T, 2, 128), -1, np.float16)
    tg2 = np.full((NCORES, BLOCKS, OV2_T, 2, 128), -1, np.float16)

    gidx[core[selm], block[selm], Tm, lanem] = pair[selm]

    e1 = oidx[lvl1]
    t1 = q[lvl1] // 128
    lane1 = q[lvl1] % 128
    T1 = 2 * WMAIN + g[e1] * OVG_T + t1
    gidx[core[e1], block[e1], T1, lane1] = pair[e1]
    tg1[core[e1], block[e1], g[e1] * OVG_T + t1, par[e1], lane1] = v[e1] % 32

    t2 = q2 // 128
    lane2 = q2 % 128
    T2 = 2 * WMAIN + GROUPS * OVG_T + t2
    gidx[core[e2], block[e2], T2, lane2] = pair[e2]
    tg2[core[e2], block[e2], t2, par[e2], lane2] = v[e2]

    # Pad slots (unused main/overflow lanes) all point at ZPAIR, the all-zero
    # row, so every block gathers exactly NIDX valid rows and num_idxs_reg is
    # the compile-time constant NIDX — no per-block count registers.
    iso = bool((in_deg == 0).any())  # any zero-in-degree node anywhere
    per_core = []
    for c in range(NCORES):
        flat = gidx[c].reshape(BLOCKS, NIDX).astype(np.int16)
        wrapped = flat.reshape(BLOCKS, NIDX // 16, 16).transpose(0, 2, 1)
        gidx_w = np.broadcast_to(
            wrapped[:, None, :, :], (BLOCKS, 8, 16, NIDX // 16)
        ).reshape(BLOCKS, 128, NIDX // 16).copy()

        tg1_dev = np.ascontiguousarray(
            tg1[c].transpose(0, 3, 1, 2).reshape(
                BLOCKS, 128, GROUPS * OVG_T * 2))
        tg2_dev = np.ascontiguousarray(
            tg2[c].transpose(0, 3, 1, 2).reshape(BLOCKS, 128, OV2_T * 2))

        nlo = c * NPC
        ind = np.zeros(NPAD, np.uint8)
        assert in_deg.max() <= 255
        ind[:NPC] = in_deg[nlo:nlo + NPC]
        # transposed + replicated across the 48 feature partitions so the
        # device can compute cj in the [feat, node] layout the blend uses
        indegT_dev = np.ascontiguousarray(
            np.broadcast_to(ind[None, :], (D, NPAD)))

        blk = np.concatenate([
            gidx_w.view(np.uint8).reshape(BLOCKS, 128, -1),
            tg1_dev.view(np.uint8).reshape(BLOCKS, 128, -1),
            tg2_dev.view(np.uint8).reshape(BLOCKS, 128, -1),
        ], axis=2)
        entry = dict(blk=np.ascontiguousarray(blk), indegT=indegT_dev)
        if iso:
            fc = np.zeros((NPAD, D), np.float32)
            fc[:NPC] = feats[nlo:nlo + NPC]
            entry["featT"] = np.ascontiguousarray(fc.T)
        per_core.append(entry)

    meta = dict(OV2_T=OV2_T, TILES=TILES, NIDX=NIDX, iso=iso, nzdeg=nzdeg)
    return xrow, per_core, meta


# ---------------------------------------------------------------------------
# Device program
# ---------------------------------------------------------------------------

def _build_program(meta):
    import concourse.tile as tile
    from concourse import bacc, mybir

    OV2_T, TILES, NIDX = meta["OV2_T"], meta["TILES"], meta["NIDX"]
    iso = meta["iso"]
    f16 = mybir.dt.float16
    f32 = mybir.dt.float32
    i16 = mybir.dt.int16
    AF = mybir.ActivationFunctionType
    OP = mybir.AluOpType

    nc = bacc.Bacc("TRN2", target_bir_lowering=False, debug=False,
                   num_devices=NCORES, num_swdge_queues=2)

    xrow = nc.dram_tensor("xrow", [PAIRS, 128], f16, kind="ExternalInput").ap()
    GIB = (NIDX // 16) * 2
    NT1 = GROUPS * OVG_T * 2
    NT2 = OV2_T * 2
    MGB = NT1 * 2
    M2B = NT2 * 2
    BLKB = GIB + MGB + M2B
    u8 = mybir.dt.uint8
    blkD = nc.dram_tensor("blk", [BLOCKS, 128, BLKB], u8, kind="ExternalInput").ap()
    indegTD = nc.dram_tensor("indegT", [D, NPAD], u8, kind="ExternalInput").ap()
    wbD = nc.dram_tensor("wb", [D, D], f32, kind="ExternalInput").ap()
    biasD = nc.dram_tensor("bias", [D, 1], f32, kind="ExternalInput").ap()
    id32D = nc.dram_tensor("id32", [128, GROUPS * 32], f16, kind="ExternalInput").ap()
    iotaD = nc.dram_tensor("iota", [128, NT1 * 32 + NT2 * 128], f16,
                           kind="ExternalInput").ap()
    if iso:
        featTD = nc.dram_tensor("featT", [D, NPAD], f32,
                                kind="ExternalInput").ap()
    outD = nc.dram_tensor("out", [D, NPAD], f16, kind="ExternalOutput").ap()

    with tile.TileContext(nc) as tc:
        with tc.tile_pool(name="const", bufs=1) as cpool, \
             tc.tile_pool(name="big", bufs=1) as bigpool:

            wb_s = cpool.tile([D, D], f32, tag="wb")
            nc.scalar.dma_start(out=wb_s[:], in_=wbD)
            bias_s = cpool.tile([D, 1], f32, tag="bias")
            nc.scalar.dma_start(out=bias_s[:], in_=biasD)
            id32 = cpool.tile([128, GROUPS * 32], f16, tag="id32")
            nc.scalar.dma_start(out=id32[:], in_=id32D)
            iota = cpool.tile([128, NT1 * 32 + NT2 * 128], f16, tag="iota")
            nc.scalar.dma_start(out=iota[:], in_=iotaD)
            iot1 = iota[:, 0:NT1 * 32].rearrange("p (t j) -> p t j", t=NT1)
            iot2 = iota[:, NT1 * 32:].rearrange("p (t j) -> p t j", t=NT2)

            # cj in the transposed [feat, node] layout the blend uses;
            # in-degree arrives pre-replicated across the 48 feature rows
            cjT = bigpool.tile([D, NPAD], f32, tag="cjT")
            ind_s = bigpool.tile([D, NPAD], u8, tag="indT")
            nc.scalar.dma_start(out=ind_s[:], in_=indegTD)
            nc.vector.tensor_scalar_max(cjT[:], ind_s[:], 1.0)
            nc.scalar.activation(cjT[:], cjT[:], AF.Sqrt)
            nc.vector.reciprocal(cjT[:], cjT[:])
            if iso:
                mask = bigpool.tile([D, NPAD], f32, tag="mask")
                nc.vector.tensor_scalar(mask[:], ind_s[:], 0.0, None,
                                        OP.is_gt)
                nc.vector.tensor_mul(cjT[:], cjT[:], mask[:])
                featT_s = bigpool.tile([D, NPAD], f32, tag="featT")
                nc.sync.dma_start(out=featT_s[:], in_=featTD)
                fbT = bigpool.tile([D, NPAD], f32, tag="fbT")
                nc.vector.tensor_scalar(mask[:], mask[:], -1.0, 1.0,
                                        OP.mult, OP.add)  # 1 - mask
                nc.vector.tensor_mul(fbT[:], featT_s[:], mask[:])

            hT1 = bigpool.tile([D, BLOCKS * 128], f32, tag="hT1")

            # ---- gather + transposed segment-sum over 49 blocks ----------
            with tc.tile_pool(name="blk", bufs=8) as blkpool, \
                 tc.tile_pool(name="msg", bufs=5) as msgpool, \
                 tc.tile_pool(name="sm", bufs=4) as smpool, \
                 tc.tile_pool(name="ps", bufs=4, space="PSUM") as pspool, \
                 tc.tile_pool(name="aux", bufs=3, space="PSUM") as auxpool:

                CH = 512
                outT = bigpool.tile([D, BLOCKS * 128], f16, tag="outT")

                for b in range(BLOCKS):
                    blkt = blkpool.tile([128, BLKB], u8, tag="blkt")
                    # block 0 rides the Pool queue (25 ns decode vs SP's
                    # 565 ns) to shorten the first-gather latency chain
                    eng = nc.gpsimd if b == 0 else nc.sync
                    eng.dma_start(out=blkt[:], in_=blkD[b])
                    gi = blkt[:, 0:GIB].bitcast(i16)
                    tg1 = blkt[:, GIB:GIB + MGB].bitcast(f16)
                    tg2 = blkt[:, GIB + MGB:BLKB].bitcast(f16)

                    msg = msgpool.tile([128, TILES, 128], f16, tag="msg")
                    nc.gpsimd.dma_gather(
                        out_ap=msg[:],
                        in_ap=xrow,
                        idxs_ap=gi,
                        num_idxs=NIDX,
                        num_idxs_reg=NIDX,
                        elem_size=128,
                        queue_num=b % 2,
                        single_packet=False,
                    )

                    # per-slot X' = feat * rsqrt(max(out_deg, 1)); degrees
                    # for both parities ride in cols 96:98 of each row.
                    # Main tiles are parity-pure, so only the half the
                    # matmuls actually read gets scaled; overflow tiles
                    # (both halves read) get both.
                    sc32 = smpool.tile([128, TILES, 2], f32, tag="sc32")
                    if meta["nzdeg"]:
                        nc.scalar.activation(sc32[:], msg[:, :, 96:98],
                                             AF.Sqrt)
                    else:
                        nc.vector.tensor_scalar_max(
                            sc32[:], msg[:, :, 96:98], 1.0)
                        nc.scalar.activation(sc32[:], sc32[:], AF.Sqrt)
                    sc = smpool.tile([128, TILES, 2], f16, tag="sc")
                    with nc.allow_low_precision("rsqrt scale, values <= 1"):
                        nc.vector.reciprocal(sc[:], sc32[:])

                    # expand per-lane targets into one-hot matmul masks
                    oh1 = smpool.tile([128, NT1, 32], f16, tag="oh1")
                    nc.vector.tensor_tensor(
                        oh1[:], iot1,
                        tg1.unsqueeze(2).to_broadcast([128, NT1, 32]),
                        OP.is_equal)
                    oh2 = smpool.tile([128, NT2, 128], f16, tag="oh2")
                    nc.vector.tensor_tensor(
                        oh2[:], iot2,
                        tg2.unsqueeze(2).to_broadcast([128, NT2, 128]),
                        OP.is_equal)
                    W2 = 2 * WMAIN
                    # overflow tiles first: the level-2 opener matmul only
                    # needs those, so the PE stream starts while the main
                    # halves are still being scaled
                    for t0, t1, p, c0 in ((W2, TILES, 0, 0),
                                          (W2, TILES, 1, 48),
                                          (0, WMAIN, 0, 0),
                                          (WMAIN, W2, 1, 48)):
                        nt = t1 - t0
                        nc.vector.tensor_tensor(
                            msg[:, t0:t1, c0:c0 + 48],
                            msg[:, t0:t1, c0:c0 + 48],
                            sc[:, t0:t1, p:p + 1].to_broadcast([128, nt, 48]),
                            OP.mult)

                    # transposed segment-sum: psT[feat, node] += msg^T @ 1hot
                    # PSUM start/stop act on whole partition rows, so the
                    # full-width level-2 matmuls open (start) and close
                    # (stop) the accumulation; everything else accumulates
                    # in between on 32-col slices.
                    psT = pspool.tile([D, 128], f32, tag="psT")
                    T2a = 2 * WMAIN + GROUPS * OVG_T
                    nc.tensor.matmul(
                        psT[:, :], lhsT=msg[:, T2a, 0:D],
                        rhs=oh2[:, 0, :],
                        start=True, stop=False, skip_group_check=True)
                    for p, c0 in ((0, 0), (1, 48)):
                        for gg in range(GROUPS):
                            for j in range(4):
                                T = p * WMAIN + gg * 4 + j
                                nc.tensor.matmul(
                                    psT[:, 32 * gg:32 * (gg + 1)],
                                    lhsT=msg[:, T, c0:c0 + D],
                                    rhs=id32[:, 32 * gg:32 * (gg + 1)],
                                    start=False, stop=False,
                                    skip_group_check=True)
                    for gg in range(GROUPS):
                        for t in range(OVG_T):
                            T = 2 * WMAIN + gg * OVG_T + t
                            for p, c0 in ((0, 0), (1, 48)):
                                nc.tensor.matmul(
                                    psT[:, 32 * gg:32 * (gg + 1)],
                                    lhsT=msg[:, T, c0:c0 + D],
                                    rhs=oh1[:, (gg * OVG_T + t) * 2 + p, :],
                                    start=False, stop=False,
                                    skip_group_check=True)
                    for t in range(OV2_T):
                        T = T2a + t
                        for pi, (p, c0) in enumerate(((0, 0), (1, 48))):
                            if t == 0 and pi == 0:
                                continue  # issued above as the opener
                            last = (t == OV2_T - 1) and (pi == 1)
                            nc.tensor.matmul(
                                psT[:, :],
                                lhsT=msg[:, T, c0:c0 + D],
                                rhs=oh2[:, t * 2 + p, :],
                                start=False, stop=last, skip_group_check=True)

                    # h^T = agg^T * cj (+ feat^T on zero-in-degree nodes)
                    bsl = slice(b * 128, (b + 1) * 128)
                    if iso:
                        th = smpool.tile([D, 128], f32, tag="th")
                        nc.vector.tensor_tensor(th[:], psT[:], cjT[:, bsl],
                                                OP.mult)
                        nc.vector.tensor_add(hT1[:, bsl], th[:],
                                             fbT[:, bsl])
                    else:
                        nc.vector.tensor_tensor(hT1[:, bsl], psT[:],
                                                cjT[:, bsl], OP.mult)

                    # ---- linear + relu, interleaved per 4 blocks ---------
                    if b % 4 == 3 or b == BLOCKS - 1:
                        lo = (b // 4) * CH
                        hi = (b + 1) * 128
                        po = auxpool.tile([D, CH], f32, tag="aux")
                        nc.tensor.matmul(po[:, 0:hi - lo], lhsT=wb_s[:],
                                         rhs=hT1[:, lo:hi], start=True,
                                         stop=True)
                        nc.scalar.activation(outT[:, lo:hi], po[:, 0:hi - lo],
                                             AF.Relu, bias=bias_s[:, 0:1])
                        nc.sync.dma_start(out=outD[:, lo:hi],
                                          in_=outT[:, lo:hi])

                # output stays transposed [48, NPAD], written per chunk
                # inside the loop; host transposes back

    nc.compile()
    return nc


# ---------------------------------------------------------------------------
# Entry point
# ---------------------------------------------------------------------------

def kernel(features, src, dst, W, b):
    from concourse.bass_utils import run_bass_kernel_spmd

    xrow, per_core, meta = _host_prep(features, src, dst)

    key = (meta["OV2_T"], meta["iso"], meta["nzdeg"])
    if key not in _CACHE:
        _CACHE[key] = _build_program(meta)
    nc = _CACHE[key]

    Wb = np.ascontiguousarray(np.asarray(W, np.float32).T)
    bias = np.ascontiguousarray(np.asarray(b, np.float32)[:, None])
    id32 = np.zeros((128, GROUPS * 32), np.float16)
    lanes = np.arange(128)
    for gg in range(GROUPS):
        id32[lanes, gg * 32 + lanes // 4] = 1.0
    nt1 = GROUPS * OVG_T * 2
    nt2 = meta["OV2_T"] * 2
    iorow = np.concatenate([
        np.tile(np.arange(32, dtype=np.float16), nt1),
        np.tile(np.arange(128, dtype=np.float16), nt2)])
    iota = np.ascontiguousarray(
        np.broadcast_to(iorow[None, :], (128, iorow.size)))

    in_maps = []
    for c in range(NCORES):
        pc = per_core[c]
        m = {
            "xrow": xrow, "blk": pc["blk"], "indegT": pc["indegT"],
            "wb": Wb, "bias": bias, "id32": id32, "iota": iota,
        }
        if meta["iso"]:
            m["featT"] = pc["featT"]
        in_maps.append(m)

    res = run_bass_kernel_spmd(nc, in_maps, core_ids=list(range(NCORES)))
    globals()["LAST_RESULTS"] = res
    out = np.concatenate(
        [res.results[c]["out"][:, :NPC].T for c in range(NCORES)], axis=0)
    return np.ascontiguousarray(out, dtype=np.float32)

